# revision 1
# baseline (speedup 1.0000x reference)
"""Trainium2 Bass kernel for nn_CNN2DImplemented_51994874085714.

conv2d: x (16, 64, 112, 112) f32 * weight (64, 3, 3, 128) -> (16, 128, 112, 112),
3x3, pad=1, stride=1 (weight layout (C_in, kh, kw, C_out), no bias).

Sharding: data-parallel over batch - 2 images per NeuronCore on 8 cores,
weight replicated; each core computes its shard independently (no
collectives) and the host concatenates the per-core outputs.

Design (settled by interleaved same-process A/B tests on HW):
- bf16 inputs (host-converted): same 1 col/cycle PE rate as fp32r, half the
  input DMA traffic. Output stays f32 (f32->bf16 DVE drains are slow on HW;
  bf16 out bought nothing once drains moved engines). rel err ~2.5e-3.
- 6 matmuls per 4-row output block (3 K=128 row-pair taps + 3 true K=64
  dh=2 singles - K=64 halves their rhs SBUF reads, measured ~4% faster
  than zero-padded K=128 singles), N=448, 8 PSUM banks rotating. 5-matmul column-pair
  schemes lose on HW: the shifted-copy/extra-DMA traffic costs more than the
  saved matmul.
- PSUM drains on the Act engine (frees DVE), staging in bf16 (halves the
  store traffic - measured ~19% faster end-to-end; f32->bf16 on Act is
  cheap, unlike on DVE), one packed store per 28-row strip; host upcasts.
- First input DMA issues before the weight load (one fat wall DMA + DVE
  peel instead of 9 small SP DMAs); input DMAs run 3 tasks ahead; the last
  strip's store is split so only a 4-row tail remains at the end.
"""

from contextlib import ExitStack

import numpy as np

N_CORES = 8
B, C, H, W, O = 16, 64, 112, 112, 128
B_LOC = B // N_CORES
S = 28  # output rows per strip

_cache = {}


def _build_nc():
    import concourse.mybir as mybir
    import concourse.tile as tile
    from concourse import bacc

    BF16 = mybir.dt.bfloat16
    F32 = mybir.dt.float32
    COPY = mybir.ActivationFunctionType.Copy

    NS = H // S
    R = S + 3
    Wp = W + 2

    nc = bacc.Bacc("TRN2", target_bir_lowering=False, debug=False,
                   num_devices=N_CORES)
    x_d = nc.declare_dram_parameter("x", [B_LOC, C, H, W], BF16, isOutput=False)
    w_d = nc.declare_dram_parameter("weight", [C, 9, O], BF16, isOutput=False)
    o_d = nc.declare_dram_parameter("out", [B_LOC, O, H, W], BF16, isOutput=True)

    with tile.TileContext(nc) as tc, ExitStack() as ctx:
        wpool = ctx.enter_context(tc.tile_pool(name="weights", bufs=1))
        xpool = ctx.enter_context(tc.tile_pool(name="xstrips", bufs=5))
        spool = ctx.enter_context(tc.tile_pool(name="staging", bufs=4))
        ppool = ctx.enter_context(tc.tile_pool(name="psum", bufs=8, space="PSUM"))

        # wall[c, t, o] = W[c, dh, dw, o] with t = 3*dh + dw
        wall = wpool.tile([64, 9, O], BF16, tag="wall")
        # wpair[dw] = [W(0,dw); W(1,dw)], wsing[dw] = [W(2,dw); 0]
        wpair = [
            wpool.tile([128, O], BF16, tag=f"wpair{dw}", name=f"wpair{dw}")
            for dw in range(3)
        ]
        # true K=64 singles: no zero half, half the rhs SBUF reads
        wsing = [
            wpool.tile([64, O], BF16, tag=f"wsing{dw}", name=f"wsing{dw}")
            for dw in range(3)
        ]

        def load_weights():
            nc.sync.dma_start(wall[:, :, :], w_d[:, :, :])
            for dw in range(3):
                nc.vector.tensor_copy(wpair[dw][0:64, :], wall[:, dw, :])
                nc.vector.tensor_copy(wpair[dw][64:128, :], wall[:, 3 + dw, :])
                nc.vector.tensor_copy(wsing[dw][0:64, :], wall[:, 6 + dw, :])

        def dma_in(t):
            # xb p0:64 row r = x_pad row h0+r (pad cols 0/113 zeroed)
            s, img = t
            h0 = s * S
            xb = xpool.tile([128, R, Wp], BF16, tag="xs")
            nc.vector.memset(xb[0:64, :, 0], 0.0)
            nc.vector.memset(xb[0:64, :, Wp - 1], 0.0)
            r_lo = max(0, 1 - h0)
            r_hi = min(S + 2, H - h0)
            if r_lo > 0:
                nc.vector.memset(xb[0:64, 0:r_lo, :], 0.0)
            if r_hi < S + 2:
                nc.vector.memset(xb[0:64, r_hi + 1:S + 3, :], 0.0)
            nc.sync.dma_start(
                xb[0:64, r_lo:r_hi + 1, 1:W + 1],
                x_d[img, :, h0 + r_lo - 1:h0 + r_hi, :],
            )
            return xb

        def make_frames(xb):
            # dh=1 frame: p64:128 = p0:64 shifted down one row
            nc.gpsimd.tensor_copy(xb[64:128, 0:S + 2, :], xb[0:64, 1:S + 3, :])
            return xb

        def compute(t, xb, last=False):
            s, img = t
            h0 = s * S
            stg = spool.tile([O, S, W], BF16, tag="stg")
            for j in range(S // 4):
                l0 = 4 * j
                ps = ppool.tile([O, 4, W], F32, tag="ps")
                for dw in range(3):
                    nc.tensor.matmul(
                        ps[:, :, :], wpair[dw][:, :],
                        xb[:, l0:l0 + 4, dw:dw + W],
                        start=(dw == 0), stop=False,
                    )
                for dw in range(3):
                    nc.tensor.matmul(
                        ps[:, :, :], wsing[dw][:, :],
                        xb[0:64, l0 + 2:l0 + 6, dw:dw + W],
                        start=False, stop=(dw == 2),
                    )
                if last and j == S // 4 - 1:
                    # store the bulk early so only a 4-row tail remains
                    nc.sync.dma_start(o_d[img, :, h0:h0 + S - 4, :],
                                      stg[:, 0:S - 4, :])
                nc.scalar.activation(stg[:, l0:l0 + 4, :], ps[:, :, :], COPY)
            if last:
                nc.sync.dma_start(o_d[img, :, h0 + S - 4:h0 + S, :],
                                  stg[:, S - 4:S, :])
            else:
                nc.sync.dma_start(o_d[img, :, h0:h0 + S, :], stg[:, :, :])

        tasks = [(s, img) for s in range(NS) for img in range(B_LOC)]
        NT = len(tasks)

        xins = {0: dma_in(tasks[0])}
        load_weights()
        xins[1] = dma_in(tasks[1])
        xins[2] = dma_in(tasks[2])
        frames = make_frames(xins.pop(0))
        for i, t in enumerate(tasks):
            if i + 3 < NT:
                xins[i + 3] = dma_in(tasks[i + 3])
            nxt = make_frames(xins.pop(i + 1)) if i + 1 < NT else None
            compute(t, frames, last=(i + 1 == NT))
            frames = nxt

    nc.compile()
    return nc


def kernel(x: np.ndarray, weight: np.ndarray) -> np.ndarray:
    import ml_dtypes
    from concourse.bass_utils import run_bass_kernel_spmd

    if "nc" not in _cache:
        _cache["nc"] = _build_nc()
    nc = _cache["nc"]

    x = np.ascontiguousarray(np.asarray(x)).astype(ml_dtypes.bfloat16)
    w = np.ascontiguousarray(np.asarray(weight)).reshape(C, 9, O).astype(
        ml_dtypes.bfloat16)

    in_maps = [
        {"x": x[i * B_LOC:(i + 1) * B_LOC], "weight": w} for i in range(N_CORES)
    ]
    res = run_bass_kernel_spmd(nc, in_maps, list(range(N_CORES)))
    return np.concatenate(
        [np.asarray(res.results[i]["out"], dtype=np.float32)
         for i in range(N_CORES)],
        axis=0,
    )



# revision 2
# speedup vs baseline: 1.2020x; 1.2020x over previous
"""Trainium2 Bass kernel for nn_CNN2DImplemented_51994874085714.

conv2d: x (16, 64, 112, 112) f32 * weight (64, 3, 3, 128) -> (16, 128, 112, 112),
3x3, pad=1, stride=1 (weight layout (C_in, kh, kw, C_out), no bias).

Sharding: data-parallel over batch - 2 images per NeuronCore on 8 cores,
weight replicated; each core computes its shard independently (no
collectives) and the host concatenates the per-core outputs.

Design (settled by interleaved same-process A/B tests on HW):
- bf16 inputs (host-converted): same 1 col/cycle PE rate as fp32r, half the
  input DMA traffic. Output stays f32 (f32->bf16 DVE drains are slow on HW;
  bf16 out bought nothing once drains moved engines). rel err ~2.5e-3.
- 6 matmuls per 4-row output block (3 K=128 row-pair taps + 3 true K=64
  dh=2 singles - K=64 halves their rhs SBUF reads, measured ~4% faster
  than zero-padded K=128 singles), N=448, 8 PSUM banks rotating. 5-matmul column-pair
  schemes lose on HW: the shifted-copy/extra-DMA traffic costs more than the
  saved matmul.
- Output stores issued from the Act engine's HWDGE ring (qActDynamicHW),
  splitting DMA across both hardware rings (input loads stay on qSPDynamicHW).
- PSUM drains on the Act engine (frees DVE), staging in bf16 (halves the
  store traffic - measured ~19% faster end-to-end; f32->bf16 on Act is
  cheap, unlike on DVE), one packed store per 28-row strip; host upcasts.
- First input DMA issues before the weight load (one fat wall DMA + DVE
  peel instead of 9 small SP DMAs); input DMAs run 3 tasks ahead; the last
  strip's store is split so only a 4-row tail remains at the end.
"""

from contextlib import ExitStack

import numpy as np

N_CORES = 8
B, C, H, W, O = 16, 64, 112, 112, 128
B_LOC = B // N_CORES
S = 28  # output rows per strip

_cache = {}


def _build_nc():
    import concourse.mybir as mybir
    import concourse.tile as tile
    from concourse import bacc

    BF16 = mybir.dt.bfloat16
    F32 = mybir.dt.float32
    COPY = mybir.ActivationFunctionType.Copy

    NS = H // S
    R = S + 3
    Wp = W + 2

    nc = bacc.Bacc("TRN2", target_bir_lowering=False, debug=False,
                   num_devices=N_CORES)
    x_d = nc.declare_dram_parameter("x", [B_LOC, C, H, W], BF16, isOutput=False)
    w_d = nc.declare_dram_parameter("weight", [C, 9, O], BF16, isOutput=False)
    o_d = nc.declare_dram_parameter("out", [B_LOC, O, H, W], BF16, isOutput=True)

    with tile.TileContext(nc) as tc, ExitStack() as ctx:
        wpool = ctx.enter_context(tc.tile_pool(name="weights", bufs=1))
        xpool = ctx.enter_context(tc.tile_pool(name="xstrips", bufs=5))
        spool = ctx.enter_context(tc.tile_pool(name="staging", bufs=4))
        ppool = ctx.enter_context(tc.tile_pool(name="psum", bufs=8, space="PSUM"))

        # wall[c, t, o] = W[c, dh, dw, o] with t = 3*dh + dw
        wall = wpool.tile([64, 9, O], BF16, tag="wall")
        # wpair[dw] = [W(0,dw); W(1,dw)], wsing[dw] = [W(2,dw); 0]
        wpair = [
            wpool.tile([128, O], BF16, tag=f"wpair{dw}", name=f"wpair{dw}")
            for dw in range(3)
        ]
        # true K=64 singles: no zero half, half the rhs SBUF reads
        wsing = [
            wpool.tile([64, O], BF16, tag=f"wsing{dw}", name=f"wsing{dw}")
            for dw in range(3)
        ]

        def load_weights():
            nc.sync.dma_start(wall[:, :, :], w_d[:, :, :])
            for dw in range(3):
                nc.vector.tensor_copy(wpair[dw][0:64, :], wall[:, dw, :])
                nc.vector.tensor_copy(wpair[dw][64:128, :], wall[:, 3 + dw, :])
                nc.vector.tensor_copy(wsing[dw][0:64, :], wall[:, 6 + dw, :])

        def dma_in(t):
            # xb p0:64 row r = x_pad row h0+r (pad cols 0/113 zeroed)
            s, img = t
            h0 = s * S
            xb = xpool.tile([128, R, Wp], BF16, tag="xs")
            nc.vector.memset(xb[0:64, :, 0], 0.0)
            nc.vector.memset(xb[0:64, :, Wp - 1], 0.0)
            r_lo = max(0, 1 - h0)
            r_hi = min(S + 2, H - h0)
            if r_lo > 0:
                nc.vector.memset(xb[0:64, 0:r_lo, :], 0.0)
            if r_hi < S + 2:
                nc.vector.memset(xb[0:64, r_hi + 1:S + 3, :], 0.0)
            nc.sync.dma_start(
                xb[0:64, r_lo:r_hi + 1, 1:W + 1],
                x_d[img, :, h0 + r_lo - 1:h0 + r_hi, :],
            )
            return xb

        def make_frames(xb):
            # dh=1 frame: p64:128 = p0:64 shifted down one row
            nc.gpsimd.tensor_copy(xb[64:128, 0:S + 2, :], xb[0:64, 1:S + 3, :])
            return xb

        def compute(t, xb, last=False):
            s, img = t
            h0 = s * S
            stg = spool.tile([O, S, W], BF16, tag="stg")
            for j in range(S // 4):
                l0 = 4 * j
                ps = ppool.tile([O, 4, W], F32, tag="ps")
                for dw in range(3):
                    nc.tensor.matmul(
                        ps[:, :, :], wpair[dw][:, :],
                        xb[:, l0:l0 + 4, dw:dw + W],
                        start=(dw == 0), stop=False,
                    )
                for dw in range(3):
                    nc.tensor.matmul(
                        ps[:, :, :], wsing[dw][:, :],
                        xb[0:64, l0 + 2:l0 + 6, dw:dw + W],
                        start=False, stop=(dw == 2),
                    )
                if last and j == S // 4 - 1:
                    # store the bulk early so only a 4-row tail remains
                    nc.scalar.dma_start(o_d[img, :, h0:h0 + S - 4, :],
                                        stg[:, 0:S - 4, :])
                nc.scalar.activation(stg[:, l0:l0 + 4, :], ps[:, :, :], COPY)
            if last:
                nc.scalar.dma_start(o_d[img, :, h0 + S - 4:h0 + S, :],
                                    stg[:, S - 4:S, :])
            else:
                nc.scalar.dma_start(o_d[img, :, h0:h0 + S, :], stg[:, :, :])

        tasks = [(s, img) for s in range(NS) for img in range(B_LOC)]
        NT = len(tasks)

        xins = {0: dma_in(tasks[0])}
        load_weights()
        xins[1] = dma_in(tasks[1])
        xins[2] = dma_in(tasks[2])
        frames = make_frames(xins.pop(0))
        for i, t in enumerate(tasks):
            if i + 3 < NT:
                xins[i + 3] = dma_in(tasks[i + 3])
            nxt = make_frames(xins.pop(i + 1)) if i + 1 < NT else None
            compute(t, frames, last=(i + 1 == NT))
            frames = nxt

    nc.compile()
    return nc


def kernel(x: np.ndarray, weight: np.ndarray) -> np.ndarray:
    import ml_dtypes
    from concourse.bass_utils import run_bass_kernel_spmd

    if "nc" not in _cache:
        _cache["nc"] = _build_nc()
    nc = _cache["nc"]

    x = np.ascontiguousarray(np.asarray(x)).astype(ml_dtypes.bfloat16)
    w = np.ascontiguousarray(np.asarray(weight)).reshape(C, 9, O).astype(
        ml_dtypes.bfloat16)

    in_maps = [
        {"x": x[i * B_LOC:(i + 1) * B_LOC], "weight": w} for i in range(N_CORES)
    ]
    res = run_bass_kernel_spmd(nc, in_maps, list(range(N_CORES)))
    return np.concatenate(
        [np.asarray(res.results[i]["out"], dtype=np.float32)
         for i in range(N_CORES)],
        axis=0,
    )



# revision 3
# speedup vs baseline: 1.2062x; 1.0035x over previous
"""Trainium2 Bass kernel for nn_CNN2DImplemented_51994874085714.

conv2d: x (16, 64, 112, 112) f32 * weight (64, 3, 3, 128) -> (16, 128, 112, 112),
3x3, pad=1, stride=1 (weight layout (C_in, kh, kw, C_out), no bias).

Sharding: data-parallel over batch - 2 images per NeuronCore on 8 cores,
weight replicated; each core computes its shard independently (no
collectives) and the host concatenates the per-core outputs.

Design (settled by interleaved same-process A/B tests on HW):
- bf16 inputs (host-converted): same 1 col/cycle PE rate as fp32r, half the
  input DMA traffic. Output stays f32 (f32->bf16 DVE drains are slow on HW;
  bf16 out bought nothing once drains moved engines). rel err ~2.5e-3.
- 6 matmuls per 4-row output block (3 K=128 row-pair taps + 3 true K=64
  dh=2 singles - K=64 halves their rhs SBUF reads, measured ~4% faster
  than zero-padded K=128 singles), N=448, 8 PSUM banks rotating. 5-matmul column-pair
  schemes lose on HW: the shifted-copy/extra-DMA traffic costs more than the
  saved matmul.
- Output stores alternate between the two HWDGE rings (qActDynamicHW /
  qSPDynamicHW) per task, balancing DMA bytes across both hardware rings.
- PSUM drains on the Act engine (frees DVE), staging in bf16 (halves the
  store traffic - measured ~19% faster end-to-end; f32->bf16 on Act is
  cheap, unlike on DVE), one packed store per 28-row strip; host upcasts.
- First input DMA issues before the weight load (one fat wall DMA + DVE
  peel instead of 9 small SP DMAs); input DMAs run 3 tasks ahead; the last
  strip's store is split so only a 4-row tail remains at the end.
"""

from contextlib import ExitStack

import numpy as np

N_CORES = 8
B, C, H, W, O = 16, 64, 112, 112, 128
B_LOC = B // N_CORES
S = 28  # output rows per strip

_cache = {}


def _build_nc():
    import concourse.mybir as mybir
    import concourse.tile as tile
    from concourse import bacc

    BF16 = mybir.dt.bfloat16
    F32 = mybir.dt.float32
    COPY = mybir.ActivationFunctionType.Copy

    NS = H // S
    R = S + 3
    Wp = W + 2

    nc = bacc.Bacc("TRN2", target_bir_lowering=False, debug=False,
                   num_devices=N_CORES)
    x_d = nc.declare_dram_parameter("x", [B_LOC, C, H, W], BF16, isOutput=False)
    w_d = nc.declare_dram_parameter("weight", [C, 9, O], BF16, isOutput=False)
    o_d = nc.declare_dram_parameter("out", [B_LOC, O, H, W], BF16, isOutput=True)

    with tile.TileContext(nc) as tc, ExitStack() as ctx:
        wpool = ctx.enter_context(tc.tile_pool(name="weights", bufs=1))
        xpool = ctx.enter_context(tc.tile_pool(name="xstrips", bufs=5))
        spool = ctx.enter_context(tc.tile_pool(name="staging", bufs=4))
        ppool = ctx.enter_context(tc.tile_pool(name="psum", bufs=8, space="PSUM"))

        # wall[c, t, o] = W[c, dh, dw, o] with t = 3*dh + dw
        wall = wpool.tile([64, 9, O], BF16, tag="wall")
        # wpair[dw] = [W(0,dw); W(1,dw)], wsing[dw] = [W(2,dw); 0]
        wpair = [
            wpool.tile([128, O], BF16, tag=f"wpair{dw}", name=f"wpair{dw}")
            for dw in range(3)
        ]
        # true K=64 singles: no zero half, half the rhs SBUF reads
        wsing = [
            wpool.tile([64, O], BF16, tag=f"wsing{dw}", name=f"wsing{dw}")
            for dw in range(3)
        ]

        def load_weights():
            nc.sync.dma_start(wall[:, :, :], w_d[:, :, :])
            for dw in range(3):
                nc.vector.tensor_copy(wpair[dw][0:64, :], wall[:, dw, :])
                nc.vector.tensor_copy(wpair[dw][64:128, :], wall[:, 3 + dw, :])
                nc.vector.tensor_copy(wsing[dw][0:64, :], wall[:, 6 + dw, :])

        def dma_in(t):
            # xb p0:64 row r = x_pad row h0+r (pad cols 0/113 zeroed)
            s, img = t
            h0 = s * S
            xb = xpool.tile([128, R, Wp], BF16, tag="xs")
            nc.vector.memset(xb[0:64, :, 0], 0.0)
            nc.vector.memset(xb[0:64, :, Wp - 1], 0.0)
            r_lo = max(0, 1 - h0)
            r_hi = min(S + 2, H - h0)
            if r_lo > 0:
                nc.vector.memset(xb[0:64, 0:r_lo, :], 0.0)
            if r_hi < S + 2:
                nc.vector.memset(xb[0:64, r_hi + 1:S + 3, :], 0.0)
            nc.sync.dma_start(
                xb[0:64, r_lo:r_hi + 1, 1:W + 1],
                x_d[img, :, h0 + r_lo - 1:h0 + r_hi, :],
            )
            return xb

        def make_frames(xb):
            # dh=1 frame: p64:128 = p0:64 shifted down one row
            nc.gpsimd.tensor_copy(xb[64:128, 0:S + 2, :], xb[0:64, 1:S + 3, :])
            return xb

        def compute(t, xb, last=False):
            s, img = t
            h0 = s * S
            # alternate stores between the two HWDGE rings (qActDynamicHW /
            # qSPDynamicHW) to balance DMA bytes across them
            sq = nc.scalar if (2 * s + img) % 2 == 0 else nc.sync
            stg = spool.tile([O, S, W], BF16, tag="stg")
            for j in range(S // 4):
                l0 = 4 * j
                ps = ppool.tile([O, 4, W], F32, tag="ps")
                for dw in range(3):
                    nc.tensor.matmul(
                        ps[:, :, :], wpair[dw][:, :],
                        xb[:, l0:l0 + 4, dw:dw + W],
                        start=(dw == 0), stop=False,
                    )
                for dw in range(3):
                    nc.tensor.matmul(
                        ps[:, :, :], wsing[dw][:, :],
                        xb[0:64, l0 + 2:l0 + 6, dw:dw + W],
                        start=False, stop=(dw == 2),
                    )
                if last and j == S // 4 - 1:
                    # store the bulk early so only a 4-row tail remains
                    sq.dma_start(o_d[img, :, h0:h0 + S - 4, :],
                                 stg[:, 0:S - 4, :])
                nc.scalar.activation(stg[:, l0:l0 + 4, :], ps[:, :, :], COPY)
            if last:
                sq.dma_start(o_d[img, :, h0 + S - 4:h0 + S, :],
                             stg[:, S - 4:S, :])
            else:
                sq.dma_start(o_d[img, :, h0:h0 + S, :], stg[:, :, :])

        tasks = [(s, img) for s in range(NS) for img in range(B_LOC)]
        NT = len(tasks)

        xins = {0: dma_in(tasks[0])}
        load_weights()
        xins[1] = dma_in(tasks[1])
        xins[2] = dma_in(tasks[2])
        frames = make_frames(xins.pop(0))
        for i, t in enumerate(tasks):
            if i + 3 < NT:
                xins[i + 3] = dma_in(tasks[i + 3])
            nxt = make_frames(xins.pop(i + 1)) if i + 1 < NT else None
            compute(t, frames, last=(i + 1 == NT))
            frames = nxt

    nc.compile()
    return nc


def kernel(x: np.ndarray, weight: np.ndarray) -> np.ndarray:
    import ml_dtypes
    from concourse.bass_utils import run_bass_kernel_spmd

    if "nc" not in _cache:
        _cache["nc"] = _build_nc()
    nc = _cache["nc"]

    x = np.ascontiguousarray(np.asarray(x)).astype(ml_dtypes.bfloat16)
    w = np.ascontiguousarray(np.asarray(weight)).reshape(C, 9, O).astype(
        ml_dtypes.bfloat16)

    in_maps = [
        {"x": x[i * B_LOC:(i + 1) * B_LOC], "weight": w} for i in range(N_CORES)
    ]
    res = run_bass_kernel_spmd(nc, in_maps, list(range(N_CORES)))
    return np.concatenate(
        [np.asarray(res.results[i]["out"], dtype=np.float32)
         for i in range(N_CORES)],
        axis=0,
    )

